# revision 1
# baseline (speedup 1.0000x reference)
"""BasisCustBiLSTM Trainium2 kernel (col-tiled).

Host: metadata MLP -> c_batch; per-sample mixed input projections XP (BLAS);
per-core packing. Device (8 cores = 4 sample-groups x 2 directions): the
recurrence in basis form on PE with PSUM accumulation over the 8 bases.
PE column-tiling: the 512 cells are split into 4 blocks of 128; block b's
gate columns are computed by PE column-group b (tile_position (0, 32b),
PSUM partitions 32b..32b+7), so the four 33-matmul accumulation chains run
concurrently in the 128x128 array (stationary M=8 per group). The
epilogue runs per block on partition stripe 32b; gate order per block is
[i|f|o|g] so one sigmoid covers 3 gate groups.
"""

import sys

for p in ("/opt/trn_rl_repo",):
    if p not in sys.path:
        sys.path.insert(0, p)

import numpy as np
import ml_dtypes

B, T, I, C = 32, 256, 512, 512
G = 4 * C
NB, EMB, KQ = 8, 64, 64
NCORES = 8
BL = 8
KT = C // 128
NBLK = 4             # cell blocks == PE column groups

bf16 = ml_dtypes.bfloat16

_CACHE = {}


def _build_program(TSTEPS=T):
    import concourse.bass as bass
    import concourse.mybir as mybir
    from concourse import bacc, tile

    dt = mybir.dt
    AF = mybir.ActivationFunctionType

    nc = bacc.Bacc(None, target_bir_lowering=False)

    wt_d = nc.dram_tensor("wt", [128, KT * NB * G], dt.bfloat16, kind="ExternalInput")
    xp_d = nc.dram_tensor("xp", [T, BL, G], dt.bfloat16, kind="ExternalInput")
    cb_d = nc.dram_tensor("cb", [128, NB * KT * BL], dt.bfloat16, kind="ExternalInput")
    vt_d = nc.dram_tensor("vt", [128, T], dt.float32, kind="ExternalInput")
    id_d = nc.dram_tensor("id8", [128, BL], dt.bfloat16, kind="ExternalInput")
    ho_d = nc.dram_tensor("ho", [T, BL, C], dt.float32, kind="ExternalOutput")

    with tile.TileContext(nc) as tc:
        with (
            tc.tile_pool(name="wt", bufs=1) as wt_pool,
            tc.tile_pool(name="const", bufs=1) as const_pool,
            tc.tile_pool(name="state", bufs=1) as state_pool,
            tc.tile_pool(name="xp", bufs=3) as xp_pool,
            tc.tile_pool(name="hn", bufs=12) as hn_pool,
            tc.tile_pool(name="scr", bufs=2) as scr_pool,
            tc.tile_pool(name="ps", bufs=1, space="PSUM") as ps_pool,
            tc.tile_pool(name="pst", bufs=4, space="PSUM") as pst_pool,
        ):
            wt = wt_pool.tile([128, KT * NB * G], dt.bfloat16)
            stripe = KT * NB * G // 8
            for j in range(8):
                nc.gpsimd.dma_start(
                    wt[:, j * stripe:(j + 1) * stripe],
                    wt_d[:, j * stripe:(j + 1) * stripe],
                )

            cb = const_pool.tile([128, NB * KT * BL], dt.bfloat16, tag="cb")
            nc.gpsimd.dma_start(cb[:], cb_d[:])
            vt = const_pool.tile([128, T], dt.float32, tag="vt")
            nc.gpsimd.dma_start(vt[:], vt_d[:])
            id8 = const_pool.tile([128, BL], dt.bfloat16, tag="id8")
            nc.gpsimd.dma_start(id8[:], id_d[:])

            hT = state_pool.tile([128, KT * BL], dt.bfloat16, tag="hT")
            nc.vector.memset(hT[:], 0)
            cst = state_pool.tile([128, 128], dt.float32, tag="cst")
            nc.vector.memset(cst[:], 0)

            for it in range(TSTEPS):
                xpt = xp_pool.tile([BL, G], dt.bfloat16, tag="xpt")
                nc.gpsimd.dma_start(xpt[:], xp_d[it, :, :])

                hn = []
                for n in range(NB):
                    t_ = hn_pool.tile([128, KT * BL], dt.bfloat16, tag="hn")
                    nc.vector.tensor_mul(
                        t_[:], hT[:], cb[:, n * KT * BL:(n + 1) * KT * BL]
                    )
                    hn.append(t_)

                # gates: block b in PSUM partitions 32b..32b+7, bank b.
                gates = ps_pool.tile([128, G], dt.float32, tag="gates")
                for b in range(NBLK):
                    pb = 32 * b
                    nc.tensor.matmul(
                        gates[pb:pb + BL, b * 512:(b + 1) * 512],
                        id8[0:BL, :], xpt[:, b * 512:(b + 1) * 512],
                        start=True, stop=False,
                        tile_position=(0, pb),
                    )
                for n in range(NB):
                    for kt in range(KT):
                        col0 = (kt * NB + n) * G
                        last = (n == NB - 1 and kt == KT - 1)
                        for b in range(NBLK):
                            pb = 32 * b
                            nc.tensor.matmul(
                                gates[pb:pb + BL, b * 512:(b + 1) * 512],
                                hn[n][:, kt * BL:(kt + 1) * BL],
                                wt[:, col0 + b * 512:col0 + (b + 1) * 512],
                                start=False, stop=last,
                                tile_position=(0, pb),
                            )

                sigs = scr_pool.tile([128, 384], dt.float32, tag="sigs")
                gg = scr_pool.tile([128, 128], dt.float32, tag="gg")
                t1 = scr_pool.tile([128, 128], dt.float32, tag="t1")
                t2 = scr_pool.tile([128, 128], dt.float32, tag="t2")
                tc_ = scr_pool.tile([128, 128], dt.float32, tag="tc")
                hm = scr_pool.tile([128, 128], dt.float32, tag="hm")
                hbf = scr_pool.tile([128, 128], dt.bfloat16, tag="hbf")
                for b in range(NBLK):
                    pb = 32 * b
                    sl = slice(pb, pb + BL)
                    g0 = b * 512
                    nc.scalar.activation(
                        sigs[sl, :], gates[sl, g0:g0 + 384], AF.Sigmoid)
                    nc.scalar.activation(
                        gg[sl, :], gates[sl, g0 + 384:g0 + 512], AF.Tanh)
                    nc.vector.tensor_mul(t1[sl, :], sigs[sl, 128:256], cst[sl, :])
                    nc.vector.tensor_mul(t2[sl, :], sigs[sl, 0:128], gg[sl, :])
                    nc.vector.tensor_add(t1[sl, :], t1[sl, :], t2[sl, :])
                    nc.vector.tensor_scalar_mul(
                        cst[sl, :], t1[sl, :], vt[sl, it:it + 1])
                    nc.scalar.activation(tc_[sl, :], cst[sl, :], AF.Tanh)
                    nc.vector.tensor_mul(hm[sl, :], sigs[sl, 256:384], tc_[sl, :])
                    nc.vector.tensor_scalar_mul(
                        hm[sl, :], hm[sl, :], vt[sl, it:it + 1])
                    nc.gpsimd.dma_start(
                        ho_d[it, :, b * 128:(b + 1) * 128], hm[sl, :])

                nc.vector.tensor_copy(hbf[:], hm[:])
                for b in range(NBLK):
                    pb = 32 * b
                    tp = pst_pool.tile([128, BL], dt.bfloat16, tag="tp")
                    nc.tensor.transpose(tp[:], hbf[pb:pb + BL, :], id8[pb:pb + BL, :], tile_position=(pb, 0))
                    nc.vector.tensor_copy(hT[:, b * BL:(b + 1) * BL], tp[:])

    nc.finalize()
    return nc


def _host_prep(x, mask, meta_author, meta_century, emb_author, emb_century,
               P_W1, P_b1, P_W2, W_ih, W_hh, b, W_ih_rev, W_hh_rev, b_rev):
    f32 = np.float32
    x = np.asarray(x, f32)
    mask = np.asarray(mask)
    q = np.concatenate(
        [np.asarray(emb_author, f32)[np.asarray(meta_author)],
         np.asarray(emb_century, f32)[np.asarray(meta_century)]], axis=1)
    h1 = np.tanh(q @ np.asarray(P_W1, f32) + np.asarray(P_b1, f32))
    logits = h1 @ np.asarray(P_W2, f32)
    e = np.exp(logits - logits.max(axis=1, keepdims=True))
    c_batch = (e / e.sum(axis=1, keepdims=True)).astype(f32)

    lengths = mask.astype(np.int64).sum(axis=1)
    t = np.arange(T)
    valid_f = (t[None, :] < lengths[:, None]).astype(f32)
    valid_r = ((T - t)[None, :] <= lengths[:, None]).astype(f32)

    def xproj(Wb, bb, xs):
        Wm = np.tensordot(c_batch, np.asarray(Wb, f32), axes=([1], [0]))
        bm = c_batch @ np.asarray(bb, f32)
        out = np.empty((B, T, G), f32)
        for i in range(B):
            np.matmul(xs[i], Wm[i].T, out=out[i])
        out += bm[:, None, :]
        # natural [i,f,g,o] x 512 -> per-block [blk, (i,f,o,g), 128]
        out = out.reshape(B, T, 4, 512)[:, :, [0, 1, 3, 2], :]
        out = out.reshape(B, T, 4, 4, 128).transpose(0, 1, 3, 2, 4)
        return np.ascontiguousarray(out.reshape(B, T, G))

    x_rev = x[:, ::-1]
    XP_f = xproj(W_ih, b, x)
    XP_r = xproj(W_ih_rev, b_rev, x_rev)

    def pack_w(Whh):
        w = np.asarray(Whh, f32)                         # [NB, G, C]
        w = w.reshape(NB, 4, 512, C)[:, [0, 1, 3, 2]]    # gate order i,f,o,g
        w = w.reshape(NB, 4, 4, 128, KT, 128)            # [n, t, blk, c, kt, p]
        w = w.transpose(5, 4, 0, 2, 1, 3)                # [p, kt, n, blk, t, c]
        return np.ascontiguousarray(w.reshape(128, KT * NB * G)).astype(bf16)

    wt_f = pack_w(W_hh)
    wt_r = pack_w(W_hh_rev)
    id8 = np.zeros((128, BL), dtype=bf16)
    for blk in range(NBLK):
        id8[32 * blk:32 * blk + BL] = np.eye(BL, dtype=bf16)

    in_maps = []
    for core in range(NCORES):
        grp, rev = core % 4, core >= 4
        sl = slice(grp * BL, (grp + 1) * BL)
        cbarr = np.zeros((128, NB, KT, BL), f32)
        cbarr[:] = c_batch[sl].T[None, :, None, :]
        v = (valid_r if rev else valid_f)[sl]            # [BL, T]
        vrep = np.zeros((128, T), f32)
        for blk in range(NBLK):
            vrep[32 * blk:32 * blk + BL] = v
        in_maps.append({
            "wt": wt_r if rev else wt_f,
            "xp": np.ascontiguousarray(
                (XP_r if rev else XP_f)[sl].transpose(1, 0, 2)).astype(bf16),
            "cb": cbarr.reshape(128, NB * KT * BL).astype(bf16),
            "vt": vrep,
            "id8": id8,
        })
    return in_maps


def _assemble(results):
    out = np.empty((B, T, 2 * C), np.float32)
    for core in range(NCORES):
        grp, rev = core % 4, core >= 4
        ho = results[core]["ho"]                         # [T, BL, C]
        hbt = ho.transpose(1, 0, 2)
        sl = slice(grp * BL, (grp + 1) * BL)
        if rev:
            out[sl, :, C:] = hbt[:, ::-1]
        else:
            out[sl, :, :C] = hbt
    return out


def kernel(**inputs):
    from concourse.bass_utils import run_bass_kernel_spmd

    in_maps = _host_prep(**inputs)
    if "nc" not in _CACHE:
        _CACHE["nc"] = _build_program()
    res = run_bass_kernel_spmd(_CACHE["nc"], in_maps, list(range(NCORES)))
    return _assemble(res.results)



# revision 4
# speedup vs baseline: 1.5587x; 1.5587x over previous
"""BasisCustBiLSTM Trainium2 kernel — cross-core-sharded recurrence.

Host: metadata MLP -> c_batch; per-sample mixed input projections XP
(BLAS); per-core packing. Device: 8 cores = 2 directions x 4 cell-blocks
of 128 cells. Each core computes the gates for its 128 cells for ALL 32
samples (M=32 stationary columns; the basis-weight stream is shared by
the whole batch, 4x less streaming than group-sharded layouts), then
exchanges its h-block with its 3 same-direction peers every step via
remote SBUF-to-SBUF DMA (XOR-relative routing, lane-merged broadcasts).

Per step: 4 col-group PE chains (one per 32-cell quarter, 128 gate cols
[i|f|o|g], 33 matmuls N=128 each, concurrent); fused epilogue over
[128 = 4 quarters x 32 samples, 32] tiles; 4 concurrent 32x32 PE
transposes -> hT block; broadcast to quad peers; 8 DVE muls build the
c-scaled stationaries (A-trick) for the next step.

Tile's scheduler cannot model remote semaphores, so the program is built
without the cross-core waits; they are attached to nofuse NOPs after
scheduling, with no-sync edges enforcing engine order.
"""

import sys

for p in ("/opt/trn_rl_repo",):
    if p not in sys.path:
        sys.path.insert(0, p)

import numpy as np
import ml_dtypes

B, T, I, C = 32, 256, 512, 512
G = 4 * C
NB, EMB, KQ = 8, 64, 64
NCORES = 8
OFT = (0, 512, 1536, 1024)  # gate-type col offsets in [i|f|g|o] space for [i|f|o|g] packing

bf16 = ml_dtypes.bfloat16

_CACHE = {}


def _build_program(TSTEPS=T):
    import concourse.bass as bass
    import concourse.mybir as mybir
    from concourse import bacc, tile
    from concourse.tile_rust import add_dep_helper

    dt = mybir.dt
    AF = mybir.ActivationFunctionType

    nc = bacc.Bacc(None, target_bir_lowering=False)

    wt_d = nc.dram_tensor("wt", [128, NB * 4 * 512], dt.bfloat16, kind="ExternalInput")
    xp_d = nc.dram_tensor("xp", [TSTEPS, 32, 512], dt.bfloat16, kind="ExternalInput")
    cb_d = nc.dram_tensor("cb", [128, NB * 128], dt.bfloat16, kind="ExternalInput")
    vt_d = nc.dram_tensor("vt", [128, TSTEPS], dt.float32, kind="ExternalInput")
    id_d = nc.dram_tensor("idq", [128, 32], dt.bfloat16, kind="ExternalInput")
    ho_d = nc.dram_tensor("ho", [128, TSTEPS * 32], dt.bfloat16, kind="ExternalOutput")

    post_waits = []

    with tile.TileContext(nc) as tc:
        with (
            tc.tile_pool(name="wt", bufs=1) as wt_pool,
            tc.tile_pool(name="const", bufs=1) as const_pool,
            tc.tile_pool(name="state", bufs=1) as state_pool,
            tc.tile_pool(name="xp", bufs=3) as xp_pool,
            tc.tile_pool(name="hn", bufs=2) as hn_pool,
            tc.tile_pool(name="hn0", bufs=1) as hn0_pool,
            tc.tile_pool(name="scr", bufs=2) as scr_pool,
            tc.tile_pool(name="ps", bufs=2, space="PSUM") as ps_pool,
            tc.tile_pool(name="pst", bufs=2, space="PSUM") as pst_pool,
        ):
            wt = wt_pool.tile([128, NB * 4 * 512], dt.bfloat16)
            stripe = NB * 4 * 512 // 8
            for j in range(8):
                nc.sync.dma_start(
                    wt[:, j * stripe:(j + 1) * stripe],
                    wt_d[:, j * stripe:(j + 1) * stripe],
                )

            cb = const_pool.tile([128, NB * 128], dt.bfloat16, tag="cb")
            nc.sync.dma_start(cb[:], cb_d[:])
            vt = const_pool.tile([128, TSTEPS], dt.float32, tag="vt")
            nc.sync.dma_start(vt[:], vt_d[:])
            idq = const_pool.tile([128, 32], dt.bfloat16, tag="idq")
            nc.sync.dma_start(idq[:], id_d[:])

            hosb = state_pool.tile([128, TSTEPS * 32], dt.bfloat16, tag="hosb")
            land = [state_pool.tile([128, 128], dt.bfloat16, name=f"land{i}")
                    for i in range(3)]
            gc = state_pool.tile([128, 64], dt.float32, tag="gc")  # [tanh(g) | cst]
            nc.vector.memset(gc[:], 0)

            hn0 = []
            for n in range(NB):
                t_ = hn0_pool.tile([128, 128], dt.bfloat16, name=f"hn0_{n}")
                nc.vector.memset(t_[:], 0)
                hn0.append(t_)

            rsem = nc.alloc_semaphore("rsem")
            lsem = nc.alloc_semaphore("lsem")
            bsem = nc._bir_kernel_barrier_sem
            nc._bir_kernel_barrier_sem_replica_groups.append(set(range(NCORES)))

            barnop = nc.gpsimd.nop(hint="barrier", nofuse=True)
            post_waits.append((barnop, bsem, 1))

            prev_g = barnop
            prev_v = None
            hn_cur = hn0

            for t in range(TSTEPS):
                xpt = xp_pool.tile([32, 512], dt.bfloat16, tag="xpt")
                nc.sync.dma_start(xpt[:], xp_d[t, :, :])

                gates = ps_pool.tile([128, 128], dt.float32, tag="gates")
                for j in range(4):
                    nc.tensor.matmul(
                        gates[32 * j:32 * j + 32, :],
                        idq[0:32, :], xpt[:, 128 * j:128 * j + 128],
                        start=True, stop=False, tile_position=(0, 32 * j),
                    )
                for n in range(NB):
                    for dlt in range(4):
                        col0 = (n * 4 + dlt) * 512
                        last = (n == NB - 1 and dlt == 3)
                        for j in range(4):
                            nc.tensor.matmul(
                                gates[32 * j:32 * j + 32, :],
                                hn_cur[n][:, 32 * dlt:32 * dlt + 32],
                                wt[:, col0 + 128 * j:col0 + 128 * j + 128],
                                start=False, stop=last, tile_position=(0, 32 * j),
                            )

                # ---- epilogue (dense [128, *]) ----
                sigs = scr_pool.tile([128, 96], dt.float32, tag="sigs")
                nc.scalar.activation(gc[:, 0:32], gates[:, 96:128], AF.Tanh)
                nc.scalar.activation(sigs[:], gates[:, 0:96], AF.Sigmoid)
                prod = scr_pool.tile([128, 64], dt.float32, tag="prod")
                nc.vector.tensor_mul(prod[:], sigs[:, 0:64], gc[:])
                u = scr_pool.tile([128, 32], dt.float32, tag="u")
                nc.vector.tensor_add(u[:], prod[:, 0:32], prod[:, 32:64])
                nc.vector.tensor_scalar_mul(gc[:, 32:64], u[:], vt[:, t:t + 1])
                tcn = scr_pool.tile([128, 32], dt.float32, tag="tcn")
                nc.scalar.activation(tcn[:], gc[:, 32:64], AF.Tanh)
                hmb = hosb[:, 32 * t:32 * t + 32]
                hop = nc.vector.tensor_mul(hmb, sigs[:, 64:96], tcn[:])

                if t % 32 == 31 or t == TSTEPS - 1:
                    lo = (t // 32) * 32
                    nc.sync.dma_start(
                        ho_d[:, lo * 32:(t + 1) * 32],
                        hosb[:, lo * 32:(t + 1) * 32],
                    )

                if t == TSTEPS - 1:
                    break

                # ---- transpose h block -> hT [cell, sample] ----
                hTps = pst_pool.tile([128, 32], dt.bfloat16, tag="hTps")
                for q in range(4):
                    nc.tensor.transpose(
                        hTps[32 * q:32 * q + 32, :],
                        hosb[32 * q:32 * q + 32, 32 * t:32 * t + 32],
                        idq[32 * q:32 * q + 32, :],
                        tile_position=(32 * q, 32 * q),
                    )

                lnd = land[t % 3]
                if t >= 3:
                    lw = nc.vector.nop(hint=f"lw{t}", nofuse=True)
                    post_waits.append((lw, lsem, 48 * (t - 2)))
                    if prev_v is not None:
                        add_dep_helper(lw.ins, prev_v.ins, sync=False, reason="vec order")
                    prev_v = lw
                cpy = nc.vector.tensor_copy(lnd[:, 0:32], hTps[:])
                if prev_v is not None:
                    add_dep_helper(cpy.ins, prev_v.ins, sync=False, reason="vec order")
                prev_v = cpy

                for dlt in (1, 2, 3):
                    p = nc.gpsimd.remote_dma_broadcast(
                        lnd[:, 32 * dlt:32 * dlt + 32], lnd[:, 0:32],
                        remote_sem=rsem, local_sem=lsem,
                        rdests=[(0, dlt)] * 8,
                    )
                    add_dep_helper(p.ins, prev_g.ins, sync=False, reason="gp order")
                    prev_g = p
                trig = nc.gpsimd.trigger_dma(count=None)
                add_dep_helper(trig.ins, prev_g.ins, sync=False, reason="gp order")
                prev_g = trig

                hwn = nc.vector.nop(hint=f"hw{t}", nofuse=True)
                post_waits.append((hwn, rsem, 48 * (t + 1)))
                add_dep_helper(hwn.ins, prev_v.ins, sync=False, reason="vec order")
                prev_v = hwn

                hn_nxt = []
                for n in range(NB):
                    t_ = hn_pool.tile([128, 128], dt.bfloat16, tag=f"hn{n}")
                    mm = nc.vector.tensor_mul(
                        t_[:], lnd[:], cb[:, 128 * n:128 * n + 128])
                    add_dep_helper(mm.ins, prev_v.ins, sync=False, reason="vec order")
                    prev_v = mm
                    hn_nxt.append(t_)
                hn_cur = hn_nxt

    for inst, sem, val in post_waits:
        inst._wait_ge(sem, val)

    nc.finalize()
    nc.insert_bir_kernel_barrier_sem_inc()
    return nc


def _host_prep(x, mask, meta_author, meta_century, emb_author, emb_century,
               P_W1, P_b1, P_W2, W_ih, W_hh, b, W_ih_rev, W_hh_rev, b_rev,
               TSTEPS=T):
    f32 = np.float32
    x = np.asarray(x, f32)
    mask = np.asarray(mask)
    q = np.concatenate(
        [np.asarray(emb_author, f32)[np.asarray(meta_author)],
         np.asarray(emb_century, f32)[np.asarray(meta_century)]], axis=1)
    h1 = np.tanh(q @ np.asarray(P_W1, f32) + np.asarray(P_b1, f32))
    logits = h1 @ np.asarray(P_W2, f32)
    e = np.exp(logits - logits.max(axis=1, keepdims=True))
    c_batch = (e / e.sum(axis=1, keepdims=True)).astype(f32)

    lengths = mask.astype(np.int64).sum(axis=1)
    t = np.arange(T)
    valid_f = (t[None, :] < lengths[:, None]).astype(f32)
    valid_r = ((T - t)[None, :] <= lengths[:, None]).astype(f32)

    def xproj(Wb, bb, xs):
        Wm = np.tensordot(c_batch, np.asarray(Wb, f32), axes=([1], [0]))
        bm = c_batch @ np.asarray(bb, f32)
        out = np.empty((B, T, G), f32)
        for i in range(B):
            np.matmul(xs[i], Wm[i].T, out=out[i])
        out += bm[:, None, :]
        return out  # [B, T, G] natural [i|f|g|o]

    x_rev = x[:, ::-1]
    XP = [xproj(W_ih, b, x), xproj(W_ih_rev, b_rev, x_rev)]
    WHH = [np.asarray(W_hh, f32), np.asarray(W_hh_rev, f32)]
    VAL = [valid_f, valid_r]

    idq = np.zeros((128, 32), dtype=bf16)
    for qq in range(4):
        idq[32 * qq:32 * qq + 32] = np.eye(32, dtype=bf16)

    cbarr = np.zeros((128, NB, 128), f32)
    for n in range(NB):
        cbarr[:, n, :] = np.tile(c_batch[:, n], 4)[None, :]
    cbarr = cbarr.reshape(128, NB * 128).astype(bf16)

    in_maps = []
    for core in range(NCORES):
        d, a = core // 4, core % 4
        # gate col index per (j, tg, c''): OFT[tg] + 128a + 32j + c''
        gidx = np.empty((4, 4, 32), np.int64)
        for j in range(4):
            for tg in range(4):
                gidx[j, tg] = OFT[tg] + 128 * a + 32 * j + np.arange(32)
        gidx = gidx.reshape(512)

        xp = XP[d][:, :TSTEPS, gidx]                      # [B, TS, 512]
        xp = np.ascontiguousarray(xp.transpose(1, 0, 2)).astype(bf16)

        W = WHH[d]                                        # [NB, G, C]
        wt = np.empty((128, NB * 4 * 512), f32)
        for n in range(NB):
            for dlt in range(4):
                blk = a ^ dlt
                wt[:, (n * 4 + dlt) * 512:(n * 4 + dlt + 1) * 512] = \
                    W[n][gidx][:, 128 * blk:128 * blk + 128].T
        wt = wt.astype(bf16)

        vrep = np.zeros((128, TSTEPS), f32)
        for j in range(4):
            vrep[32 * j:32 * j + 32] = VAL[d][:, :TSTEPS]

        in_maps.append({
            "wt": wt,
            "xp": xp,
            "cb": cbarr,
            "vt": vrep,
            "idq": idq,
        })
    return in_maps


def _assemble(results, TSTEPS=T):
    out = np.empty((B, TSTEPS, 2 * C), np.float32)
    for core in range(NCORES):
        d, a = core // 4, core % 4
        ho = np.asarray(results[core]["ho"], np.float32)  # [128, TS*32]
        ho = ho.reshape(4, 32, TSTEPS, 32)                # [j, s, t, c']
        ho = ho.transpose(1, 2, 0, 3).reshape(32, TSTEPS, 128)  # [s, t, cell]
        if d == 1:
            ho = ho[:, ::-1]
        out[:, :, d * 512 + 128 * a: d * 512 + 128 * a + 128] = ho
    return out


def kernel(**inputs):
    from concourse.bass_utils import run_bass_kernel_spmd

    in_maps = _host_prep(**inputs)
    if "nc" not in _CACHE:
        _CACHE["nc"] = _build_program()
    res = run_bass_kernel_spmd(_CACHE["nc"], in_maps, list(range(NCORES)))
    return _assemble(res.results)


# revision 8
# speedup vs baseline: 1.8205x; 1.1680x over previous
"""BasisCustBiLSTM Trainium2 kernel — cross-core-sharded recurrence.

Host: metadata MLP -> c_batch; per-sample mixed input projections XP
(BLAS); per-core packing. Device: 8 cores = 2 directions x 4 cell-blocks
of 128 cells. Each core computes the gates for its 128 cells for ALL 32
samples (M=32 stationary columns; the basis-weight stream is shared by
the whole batch, 4x less streaming than group-sharded layouts), then
exchanges its h-block with its 3 same-direction peers every step via
remote SBUF-to-SBUF DMA (XOR-relative routing, lane-merged broadcasts).

Per step: 4 col-group PE chains (one per 32-cell quarter, 128 gate cols
[i|f|o|g], 33 matmuls N=128 each, concurrent); fused epilogue over
[128 = 4 quarters x 32 samples, 32] tiles; 4 concurrent 32x32 PE
transposes -> hT block; broadcast to quad peers; 8 DVE muls build the
c-scaled stationaries (A-trick) for the next step.

Tile's scheduler cannot model remote semaphores, so the program is built
without the cross-core waits; they are attached to nofuse NOPs after
scheduling, with no-sync edges enforcing engine order.
"""

import sys

for p in ("/opt/trn_rl_repo",):
    if p not in sys.path:
        sys.path.insert(0, p)

import numpy as np
import ml_dtypes

B, T, I, C = 32, 256, 512, 512
G = 4 * C
NB, EMB, KQ = 8, 64, 64
NCORES = 8
OFT = (0, 512, 1536, 1024)  # gate-type col offsets in [i|f|g|o] space for [i|f|o|g] packing

bf16 = ml_dtypes.bfloat16

_CACHE = {}


def _build_program(TSTEPS=T):
    import concourse.bass as bass
    import concourse.mybir as mybir
    from concourse import bacc, tile
    from concourse.tile_rust import add_dep_helper

    dt = mybir.dt
    AF = mybir.ActivationFunctionType

    nc = bacc.Bacc(None, target_bir_lowering=False)

    wt_d = nc.dram_tensor("wt", [128, NB * 4 * 512], dt.bfloat16, kind="ExternalInput")
    xp_d = nc.dram_tensor("xp", [TSTEPS, 32, 512], dt.bfloat16, kind="ExternalInput")
    cb_d = nc.dram_tensor("cb", [128, NB * 128], dt.bfloat16, kind="ExternalInput")
    vt_d = nc.dram_tensor("vt", [128, TSTEPS], dt.float32, kind="ExternalInput")
    id_d = nc.dram_tensor("idq", [128, 32], dt.bfloat16, kind="ExternalInput")
    ho_d = nc.dram_tensor("ho", [128, TSTEPS * 32], dt.bfloat16, kind="ExternalOutput")

    post_waits = []

    with tile.TileContext(nc) as tc:
        with (
            tc.tile_pool(name="wt", bufs=1) as wt_pool,
            tc.tile_pool(name="const", bufs=1) as const_pool,
            tc.tile_pool(name="state", bufs=1) as state_pool,
            tc.tile_pool(name="xp", bufs=3) as xp_pool,
            tc.tile_pool(name="hn", bufs=2) as hn_pool,
            tc.tile_pool(name="hn0", bufs=1) as hn0_pool,
            tc.tile_pool(name="scr", bufs=2) as scr_pool,
            tc.tile_pool(name="ps", bufs=2, space="PSUM") as ps_pool,
            tc.tile_pool(name="pst", bufs=2, space="PSUM") as pst_pool,
        ):
            wt = wt_pool.tile([128, NB * 4 * 512], dt.bfloat16)
            stripe = NB * 4 * 512 // 8
            for j in range(8):
                nc.sync.dma_start(
                    wt[:, j * stripe:(j + 1) * stripe],
                    wt_d[:, j * stripe:(j + 1) * stripe],
                )

            cb = const_pool.tile([128, NB * 128], dt.bfloat16, tag="cb")
            nc.sync.dma_start(cb[:], cb_d[:])
            vt = const_pool.tile([128, TSTEPS], dt.float32, tag="vt")
            nc.sync.dma_start(vt[:], vt_d[:])
            idq = const_pool.tile([128, 32], dt.bfloat16, tag="idq")
            nc.sync.dma_start(idq[:], id_d[:])

            hosb = state_pool.tile([128, TSTEPS * 32], dt.bfloat16, tag="hosb")
            land = [state_pool.tile([128, 128], dt.bfloat16, name=f"land{i}")
                    for i in range(3)]
            gc = state_pool.tile([128, 64], dt.float32, tag="gc")  # [tanh(g) | cst]
            nc.vector.memset(gc[:], 0)

            hn0 = []
            for n in range(NB):
                t_ = hn0_pool.tile([128, 128], dt.bfloat16, name=f"hn0_{n}")
                nc.vector.memset(t_[:], 0)
                hn0.append(t_)

            rsem = nc.alloc_semaphore("rsem")
            lsem = nc.alloc_semaphore("lsem")
            bsem = nc._bir_kernel_barrier_sem
            nc._bir_kernel_barrier_sem_replica_groups.append(set(range(NCORES)))

            barnop = nc.gpsimd.nop(hint="barrier", nofuse=True)
            post_waits.append((barnop, bsem, 1))

            prev_g = barnop
            prev_v = None
            hn_cur = hn0

            warm_ps = pst_pool.tile([32, 32], dt.float32, tag="warm", bufs=1)

            def warmer(dep=None):
                w = nc.tensor.matmul(
                    warm_ps[:], idq[0:32, :], idq[0:32, :],
                    start=True, stop=True, tile_position=(0, 0),
                )
                if dep is not None:
                    add_dep_helper(w.ins, dep.ins, sync=True, reason="warm spacing")
                return w

            for t in range(TSTEPS):
                xpt = xp_pool.tile([32, 512], dt.bfloat16, tag="xpt")
                nc.sync.dma_start(xpt[:], xp_d[t, :, :])

                # hoisted descriptor generation for this step's broadcasts
                # (prepare-only: no data read; trigger comes after the copy)
                if t < TSTEPS - 1:
                    lnd = land[t % 3]
                    for dlt in (1, 2, 3):
                        p = nc.gpsimd.remote_dma_broadcast(
                            lnd[:, 32 * dlt:32 * dlt + 32], lnd[:, 0:32],
                            remote_sem=rsem, local_sem=lsem,
                            rdests=[(0, dlt)] * 8,
                        )
                        add_dep_helper(p.ins, prev_g.ins, sync=False, reason="gp order")
                        prev_g = p

                gates = ps_pool.tile([128, 128], dt.float32, tag="gates")
                for j in range(4):
                    nc.tensor.matmul(
                        gates[32 * j:32 * j + 32, :],
                        idq[0:32, :], xpt[:, 128 * j:128 * j + 128],
                        start=True, stop=False, tile_position=(0, 32 * j),
                    )
                for n in range(NB):
                    for dlt in range(4):
                        col0 = (n * 4 + dlt) * 512
                        last = (n == NB - 1 and dlt == 3)
                        for j in range(4):
                            nc.tensor.matmul(
                                gates[32 * j:32 * j + 32, :],
                                hn_cur[n][:, 32 * dlt:32 * dlt + 32],
                                wt[:, col0 + 128 * j:col0 + 128 * j + 128],
                                start=False, stop=last, tile_position=(0, 32 * j),
                            )

                # ---- epilogue (dense [128, *]) ----
                sigs = scr_pool.tile([128, 96], dt.float32, tag="sigs")
                nc.scalar.activation(gc[:, 0:32], gates[:, 96:128], AF.Tanh)
                nc.scalar.activation(sigs[:], gates[:, 0:96], AF.Sigmoid)
                prod = scr_pool.tile([128, 64], dt.float32, tag="prod")
                nc.vector.tensor_mul(prod[:], sigs[:, 0:64], gc[:])
                u = scr_pool.tile([128, 32], dt.float32, tag="u")
                nc.vector.tensor_add(u[:], prod[:, 0:32], prod[:, 32:64])
                nc.vector.tensor_scalar_mul(gc[:, 32:64], u[:], vt[:, t:t + 1])
                tcn = scr_pool.tile([128, 32], dt.float32, tag="tcn")
                atc = nc.scalar.activation(tcn[:], gc[:, 32:64], AF.Tanh)
                hmb = hosb[:, 32 * t:32 * t + 32]
                hop = nc.vector.tensor_mul(hmb, sigs[:, 64:96], tcn[:])
                warmer(atc)  # keep HAM warm through the epilogue

                if t % 32 == 31 or t == TSTEPS - 1:
                    lo = (t // 32) * 32
                    nc.sync.dma_start(
                        ho_d[:, lo * 32:(t + 1) * 32],
                        hosb[:, lo * 32:(t + 1) * 32],
                    )

                if t == TSTEPS - 1:
                    break

                # ---- transpose h block -> hT [cell, sample] ----
                hTps = pst_pool.tile([128, 32], dt.bfloat16, tag="hTps")
                for q in range(4):
                    nc.tensor.transpose(
                        hTps[32 * q:32 * q + 32, :],
                        hosb[32 * q:32 * q + 32, 32 * t:32 * t + 32],
                        idq[32 * q:32 * q + 32, :],
                        tile_position=(32 * q, 32 * q),
                    )

                lnd = land[t % 3]
                if t >= 3:
                    lw = nc.vector.nop(hint=f"lw{t}", nofuse=True)
                    post_waits.append((lw, lsem, 48 * (t - 2)))
                    if prev_v is not None:
                        add_dep_helper(lw.ins, prev_v.ins, sync=False, reason="vec order")
                    prev_v = lw
                cpy = nc.vector.tensor_copy(lnd[:, 0:32], hTps[:])
                if prev_v is not None:
                    add_dep_helper(cpy.ins, prev_v.ins, sync=False, reason="vec order")
                prev_v = cpy

                trig = nc.gpsimd.trigger_dma(count=None)
                add_dep_helper(trig.ins, prev_g.ins, sync=False, reason="gp order")
                add_dep_helper(trig.ins, cpy.ins, sync=True,
                               reason="hoisted preps: fire only after h copy lands")
                prev_g = trig
                warmer(cpy)  # keep HAM warm across the exchange window

                hwn = nc.vector.nop(hint=f"hw{t}", nofuse=True)
                post_waits.append((hwn, rsem, 48 * (t + 1)))
                add_dep_helper(hwn.ins, prev_v.ins, sync=False, reason="vec order")
                prev_v = hwn

                hn_nxt = []
                for n in range(NB):
                    t_ = hn_pool.tile([128, 128], dt.bfloat16, tag=f"hn{n}")
                    mm = nc.vector.tensor_mul(
                        t_[:], lnd[:], cb[:, 128 * n:128 * n + 128])
                    add_dep_helper(mm.ins, prev_v.ins, sync=False, reason="vec order")
                    prev_v = mm
                    hn_nxt.append(t_)
                hn_cur = hn_nxt

    for inst, sem, val in post_waits:
        inst._wait_ge(sem, val)

    nc.finalize()
    nc.insert_bir_kernel_barrier_sem_inc()
    return nc


def _host_prep(x, mask, meta_author, meta_century, emb_author, emb_century,
               P_W1, P_b1, P_W2, W_ih, W_hh, b, W_ih_rev, W_hh_rev, b_rev,
               TSTEPS=T):
    f32 = np.float32
    x = np.asarray(x, f32)
    mask = np.asarray(mask)
    q = np.concatenate(
        [np.asarray(emb_author, f32)[np.asarray(meta_author)],
         np.asarray(emb_century, f32)[np.asarray(meta_century)]], axis=1)
    h1 = np.tanh(q @ np.asarray(P_W1, f32) + np.asarray(P_b1, f32))
    logits = h1 @ np.asarray(P_W2, f32)
    e = np.exp(logits - logits.max(axis=1, keepdims=True))
    c_batch = (e / e.sum(axis=1, keepdims=True)).astype(f32)

    lengths = mask.astype(np.int64).sum(axis=1)
    t = np.arange(T)
    valid_f = (t[None, :] < lengths[:, None]).astype(f32)
    valid_r = ((T - t)[None, :] <= lengths[:, None]).astype(f32)

    def xproj(Wb, bb, xs):
        Wm = np.tensordot(c_batch, np.asarray(Wb, f32), axes=([1], [0]))
        bm = c_batch @ np.asarray(bb, f32)
        out = np.empty((B, T, G), f32)
        for i in range(B):
            np.matmul(xs[i], Wm[i].T, out=out[i])
        out += bm[:, None, :]
        return out  # [B, T, G] natural [i|f|g|o]

    x_rev = x[:, ::-1]
    XP = [xproj(W_ih, b, x), xproj(W_ih_rev, b_rev, x_rev)]
    WHH = [np.asarray(W_hh, f32), np.asarray(W_hh_rev, f32)]
    VAL = [valid_f, valid_r]

    idq = np.zeros((128, 32), dtype=bf16)
    for qq in range(4):
        idq[32 * qq:32 * qq + 32] = np.eye(32, dtype=bf16)

    cbarr = np.zeros((128, NB, 128), f32)
    for n in range(NB):
        cbarr[:, n, :] = np.tile(c_batch[:, n], 4)[None, :]
    cbarr = cbarr.reshape(128, NB * 128).astype(bf16)

    in_maps = []
    for core in range(NCORES):
        d, a = core // 4, core % 4
        # gate col index per (j, tg, c''): OFT[tg] + 128a + 32j + c''
        gidx = np.empty((4, 4, 32), np.int64)
        for j in range(4):
            for tg in range(4):
                gidx[j, tg] = OFT[tg] + 128 * a + 32 * j + np.arange(32)
        gidx = gidx.reshape(512)

        xp = XP[d][:, :TSTEPS, gidx]                      # [B, TS, 512]
        xp = np.ascontiguousarray(xp.transpose(1, 0, 2)).astype(bf16)

        W = WHH[d]                                        # [NB, G, C]
        wt = np.empty((128, NB * 4 * 512), f32)
        for n in range(NB):
            for dlt in range(4):
                blk = a ^ dlt
                wt[:, (n * 4 + dlt) * 512:(n * 4 + dlt + 1) * 512] = \
                    W[n][gidx][:, 128 * blk:128 * blk + 128].T
        wt = wt.astype(bf16)

        vrep = np.zeros((128, TSTEPS), f32)
        for j in range(4):
            vrep[32 * j:32 * j + 32] = VAL[d][:, :TSTEPS]

        in_maps.append({
            "wt": wt,
            "xp": xp,
            "cb": cbarr,
            "vt": vrep,
            "idq": idq,
        })
    return in_maps


def _assemble(results, TSTEPS=T):
    out = np.empty((B, TSTEPS, 2 * C), np.float32)
    for core in range(NCORES):
        d, a = core // 4, core % 4
        ho = np.asarray(results[core]["ho"], np.float32)  # [128, TS*32]
        ho = ho.reshape(4, 32, TSTEPS, 32)                # [j, s, t, c']
        ho = ho.transpose(1, 2, 0, 3).reshape(32, TSTEPS, 128)  # [s, t, cell]
        if d == 1:
            ho = ho[:, ::-1]
        out[:, :, d * 512 + 128 * a: d * 512 + 128 * a + 128] = ho
    return out


def kernel(**inputs):
    from concourse.bass_utils import run_bass_kernel_spmd

    in_maps = _host_prep(**inputs)
    if "nc" not in _CACHE:
        _CACHE["nc"] = _build_program()
    res = run_bass_kernel_spmd(_CACHE["nc"], in_maps, list(range(NCORES)))
    return _assemble(res.results)


# revision 11
# speedup vs baseline: 1.9210x; 1.0552x over previous
"""BasisCustBiLSTM Trainium2 kernel — cross-core-sharded recurrence.

Host: metadata MLP -> c_batch; per-sample mixed input projections XP
(BLAS); per-core packing. Device: 8 cores = 2 directions x 4 cell-blocks
of 128 cells. Each core computes the gates for its 128 cells for ALL 32
samples (M=32 stationary columns; the basis-weight stream is shared by
the whole batch, 4x less streaming than group-sharded layouts), then
exchanges its h-block with its 3 same-direction peers every step via
remote SBUF-to-SBUF DMA (XOR-relative routing, lane-merged broadcasts).

Per step: 4 col-group PE chains (one per 32-cell quarter, 128 gate cols
[i|f|o|g], 33 matmuls N=128 each, concurrent); fused epilogue over
[128 = 4 quarters x 32 samples, 32] tiles; 4 concurrent 32x32 PE
transposes -> hT block; broadcast to quad peers; 8 DVE muls build the
c-scaled stationaries (A-trick) for the next step.

Tile's scheduler cannot model remote semaphores, so the program is built
without the cross-core waits; they are attached to nofuse NOPs after
scheduling, with no-sync edges enforcing engine order.
"""

import sys

for p in ("/opt/trn_rl_repo",):
    if p not in sys.path:
        sys.path.insert(0, p)

import numpy as np
import ml_dtypes

B, T, I, C = 32, 256, 512, 512
G = 4 * C
NB, EMB, KQ = 8, 64, 64
NCORES = 8
OFT = (0, 512, 1536, 1024)  # gate-type col offsets in [i|f|g|o] space for [i|f|o|g] packing

bf16 = ml_dtypes.bfloat16

_CACHE = {}


def _build_program(TSTEPS=T):
    import concourse.bass as bass
    import concourse.mybir as mybir
    from concourse import bacc, tile
    from concourse.tile_rust import add_dep_helper

    dt = mybir.dt
    AF = mybir.ActivationFunctionType

    nc = bacc.Bacc(None, target_bir_lowering=False, num_swdge_queues=4)

    wt_d = nc.dram_tensor("wt", [128, NB * 4 * 512], dt.bfloat16, kind="ExternalInput")
    xp_d = nc.dram_tensor("xp", [TSTEPS, 32, 512], dt.bfloat16, kind="ExternalInput")
    cb_d = nc.dram_tensor("cb", [128, NB * 128], dt.bfloat16, kind="ExternalInput")
    vt_d = nc.dram_tensor("vt", [128, TSTEPS], dt.float32, kind="ExternalInput")
    id_d = nc.dram_tensor("idq", [128, 32], dt.bfloat16, kind="ExternalInput")
    ho_d = nc.dram_tensor("ho", [128, TSTEPS * 32], dt.bfloat16, kind="ExternalOutput")

    post_waits = []

    with tile.TileContext(nc) as tc:
        with (
            tc.tile_pool(name="wt", bufs=1) as wt_pool,
            tc.tile_pool(name="const", bufs=1) as const_pool,
            tc.tile_pool(name="state", bufs=1) as state_pool,
            tc.tile_pool(name="xp", bufs=3) as xp_pool,
            tc.tile_pool(name="hn", bufs=2) as hn_pool,
            tc.tile_pool(name="hn0", bufs=1) as hn0_pool,
            tc.tile_pool(name="scr", bufs=2) as scr_pool,
            tc.tile_pool(name="ps", bufs=2, space="PSUM") as ps_pool,
            tc.tile_pool(name="pst", bufs=2, space="PSUM") as pst_pool,
        ):
            wt = wt_pool.tile([128, NB * 4 * 512], dt.bfloat16)
            stripe = NB * 4 * 512 // 8
            for j in range(8):
                nc.sync.dma_start(
                    wt[:, j * stripe:(j + 1) * stripe],
                    wt_d[:, j * stripe:(j + 1) * stripe],
                )

            cb = const_pool.tile([128, NB * 128], dt.bfloat16, tag="cb")
            nc.sync.dma_start(cb[:], cb_d[:])
            vt = const_pool.tile([128, TSTEPS], dt.float32, tag="vt")
            nc.sync.dma_start(vt[:], vt_d[:])
            idq = const_pool.tile([128, 32], dt.bfloat16, tag="idq")
            nc.sync.dma_start(idq[:], id_d[:])

            hosb = state_pool.tile([128, TSTEPS * 32], dt.bfloat16, tag="hosb")
            land = [state_pool.tile([128, 128], dt.bfloat16, name=f"land{i}")
                    for i in range(3)]
            gc = state_pool.tile([128, 64], dt.float32, tag="gc")  # [tanh(g) | cst]
            nc.vector.memset(gc[:], 0)

            hn0 = []
            for n in range(NB):
                t_ = hn0_pool.tile([128, 128], dt.bfloat16, name=f"hn0_{n}")
                nc.vector.memset(t_[:], 0)
                hn0.append(t_)

            rsem = nc.alloc_semaphore("rsem")
            lsem = nc.alloc_semaphore("lsem")
            bsem = nc._bir_kernel_barrier_sem
            nc._bir_kernel_barrier_sem_replica_groups.append(set(range(NCORES)))

            barnop = nc.gpsimd.nop(hint="barrier", nofuse=True)
            post_waits.append((barnop, bsem, 1))

            prev_g = barnop
            prev_v = None
            hn_cur = hn0

            warm_ps = pst_pool.tile([32, 32], dt.float32, tag="warm", bufs=1)

            def warmer(dep=None):
                w = nc.tensor.matmul(
                    warm_ps[:], idq[0:32, :], idq[0:32, :],
                    start=True, stop=True, tile_position=(0, 0),
                )
                if dep is not None:
                    add_dep_helper(w.ins, dep.ins, sync=True, reason="warm spacing")
                return w

            def emit_seed_own(gates, xpt, hn):
                """Seed matmuls + own-chunk (dlt=0) accumulation."""
                for j in range(4):
                    nc.tensor.matmul(
                        gates[32 * j:32 * j + 32, :],
                        idq[0:32, :], xpt[:, 128 * j:128 * j + 128],
                        start=True, stop=False, tile_position=(0, 32 * j),
                    )
                for n in range(NB):
                    col0 = (n * 4) * 512
                    for j in range(4):
                        nc.tensor.matmul(
                            gates[32 * j:32 * j + 32, :],
                            hn[n][:, 0:32],
                            wt[:, col0 + 128 * j:col0 + 128 * j + 128],
                            start=False, stop=False, tile_position=(0, 32 * j),
                        )

            def emit_remote(gates, hn):
                """Remote-chunk (dlt=1..3) accumulation, closing the chains."""
                for n in range(NB):
                    for dlt in (1, 2, 3):
                        col0 = (n * 4 + dlt) * 512
                        last = (n == NB - 1 and dlt == 3)
                        for j in range(4):
                            nc.tensor.matmul(
                                gates[32 * j:32 * j + 32, :],
                                hn[n][:, 32 * dlt:32 * dlt + 32],
                                wt[:, col0 + 128 * j:col0 + 128 * j + 128],
                                start=False, stop=last, tile_position=(0, 32 * j),
                            )

            # prologue: step 0 computes gates entirely from xp (hn0 == 0)
            xpt = xp_pool.tile([32, 512], dt.bfloat16, tag="xpt")
            nc.sync.dma_start(xpt[:], xp_d[0, :, :])
            gates_cur = ps_pool.tile([128, 128], dt.float32, tag="gates")
            emit_seed_own(gates_cur, xpt, hn0)

            for t in range(TSTEPS):
                # hoisted descriptor generation for this step's broadcasts
                # (prepare-only: no data read; triggers come after the copy)
                if t < TSTEPS - 1:
                    lnd = land[t % 3]
                    for qn, dlt in enumerate((1, 2, 3)):
                        p = nc.gpsimd.remote_dma_broadcast(
                            lnd[:, 32 * dlt:32 * dlt + 32], lnd[:, 0:32],
                            remote_sem=rsem, local_sem=lsem,
                            rdests=[(0, dlt)] * 8, queue_num=qn,
                        )
                        add_dep_helper(p.ins, prev_g.ins, sync=False, reason="gp order")
                        prev_g = p

                gates = gates_cur
                emit_remote(gates, hn_cur)

                # ---- epilogue (dense [128, *]) ----
                sigs = scr_pool.tile([128, 96], dt.float32, tag="sigs")
                nc.scalar.activation(gc[:, 0:32], gates[:, 96:128], AF.Tanh)
                nc.scalar.activation(sigs[:], gates[:, 0:96], AF.Sigmoid)
                prod = scr_pool.tile([128, 64], dt.float32, tag="prod")
                nc.vector.tensor_mul(prod[:], sigs[:, 0:64], gc[:])
                u = scr_pool.tile([128, 32], dt.float32, tag="u")
                nc.vector.tensor_add(u[:], prod[:, 0:32], prod[:, 32:64])
                nc.vector.tensor_scalar_mul(gc[:, 32:64], u[:], vt[:, t:t + 1])
                tcn = scr_pool.tile([128, 32], dt.float32, tag="tcn")
                atc = nc.scalar.activation(tcn[:], gc[:, 32:64], AF.Tanh)
                hmb = hosb[:, 32 * t:32 * t + 32]
                hop = nc.vector.tensor_mul(hmb, sigs[:, 64:96], tcn[:])
                warmer(atc)  # keep HAM warm through the epilogue

                if t % 32 == 31 or t == TSTEPS - 1:
                    lo = (t // 32) * 32
                    nc.sync.dma_start(
                        ho_d[:, lo * 32:(t + 1) * 32],
                        hosb[:, lo * 32:(t + 1) * 32],
                    )

                if t == TSTEPS - 1:
                    break

                # ---- transpose h block -> hT [cell, sample] ----
                hTps = pst_pool.tile([128, 32], dt.bfloat16, tag="hTps")
                for q in range(4):
                    nc.tensor.transpose(
                        hTps[32 * q:32 * q + 32, :],
                        hosb[32 * q:32 * q + 32, 32 * t:32 * t + 32],
                        idq[32 * q:32 * q + 32, :],
                        tile_position=(32 * q, 32 * q),
                    )

                lnd = land[t % 3]
                if t >= 3:
                    lw = nc.vector.nop(hint=f"lw{t}", nofuse=True)
                    post_waits.append((lw, lsem, 48 * (t - 2)))
                    if prev_v is not None:
                        add_dep_helper(lw.ins, prev_v.ins, sync=False, reason="vec order")
                    prev_v = lw
                cpy = nc.vector.tensor_copy(lnd[:, 0:32], hTps[:])
                if prev_v is not None:
                    add_dep_helper(cpy.ins, prev_v.ins, sync=False, reason="vec order")
                prev_v = cpy

                first_trig = None
                for qn in range(3):
                    trig = nc.gpsimd.trigger_dma(count=None, queue_num=qn)
                    add_dep_helper(trig.ins, prev_g.ins, sync=False, reason="gp order")
                    if first_trig is None:
                        add_dep_helper(trig.ins, cpy.ins, sync=True,
                                       reason="hoisted preps: fire after h copy lands")
                        first_trig = trig
                    prev_g = trig

                # own-chunk stationaries + next step's seeds/own matmuls run
                # during the exchange; remote stationaries after arrival.
                hn_nxt = []
                for n in range(NB):
                    t_ = hn_pool.tile([128, 128], dt.bfloat16, tag=f"hn{n}")
                    hn_nxt.append(t_)
                for n in range(NB):
                    mm = nc.vector.tensor_mul(
                        hn_nxt[n][:, 0:32], lnd[:, 0:32], cb[:, 128 * n:128 * n + 32])
                    add_dep_helper(mm.ins, prev_v.ins, sync=False, reason="vec order")
                    prev_v = mm

                xpt = xp_pool.tile([32, 512], dt.bfloat16, tag="xpt")
                nc.sync.dma_start(xpt[:], xp_d[t + 1, :, :])
                gates_cur = ps_pool.tile([128, 128], dt.float32, tag="gates")
                emit_seed_own(gates_cur, xpt, hn_nxt)

                hwn = nc.vector.nop(hint=f"hw{t}", nofuse=True)
                post_waits.append((hwn, rsem, 48 * (t + 1)))
                add_dep_helper(hwn.ins, prev_v.ins, sync=False, reason="vec order")
                prev_v = hwn
                for n in range(NB):
                    mm = nc.vector.tensor_mul(
                        hn_nxt[n][:, 32:128], lnd[:, 32:128],
                        cb[:, 128 * n + 32:128 * n + 128])
                    add_dep_helper(mm.ins, prev_v.ins, sync=False, reason="vec order")
                    prev_v = mm
                hn_cur = hn_nxt

    for inst, sem, val in post_waits:
        inst._wait_ge(sem, val)

    nc.finalize()
    nc.insert_bir_kernel_barrier_sem_inc()
    return nc


def _host_prep(x, mask, meta_author, meta_century, emb_author, emb_century,
               P_W1, P_b1, P_W2, W_ih, W_hh, b, W_ih_rev, W_hh_rev, b_rev,
               TSTEPS=T):
    f32 = np.float32
    x = np.asarray(x, f32)
    mask = np.asarray(mask)
    q = np.concatenate(
        [np.asarray(emb_author, f32)[np.asarray(meta_author)],
         np.asarray(emb_century, f32)[np.asarray(meta_century)]], axis=1)
    h1 = np.tanh(q @ np.asarray(P_W1, f32) + np.asarray(P_b1, f32))
    logits = h1 @ np.asarray(P_W2, f32)
    e = np.exp(logits - logits.max(axis=1, keepdims=True))
    c_batch = (e / e.sum(axis=1, keepdims=True)).astype(f32)

    lengths = mask.astype(np.int64).sum(axis=1)
    t = np.arange(T)
    valid_f = (t[None, :] < lengths[:, None]).astype(f32)
    valid_r = ((T - t)[None, :] <= lengths[:, None]).astype(f32)

    def xproj(Wb, bb, xs):
        Wm = np.tensordot(c_batch, np.asarray(Wb, f32), axes=([1], [0]))
        bm = c_batch @ np.asarray(bb, f32)
        out = np.empty((B, T, G), f32)
        for i in range(B):
            np.matmul(xs[i], Wm[i].T, out=out[i])
        out += bm[:, None, :]
        return out  # [B, T, G] natural [i|f|g|o]

    x_rev = x[:, ::-1]
    XP = [xproj(W_ih, b, x), xproj(W_ih_rev, b_rev, x_rev)]
    WHH = [np.asarray(W_hh, f32), np.asarray(W_hh_rev, f32)]
    VAL = [valid_f, valid_r]

    idq = np.zeros((128, 32), dtype=bf16)
    for qq in range(4):
        idq[32 * qq:32 * qq + 32] = np.eye(32, dtype=bf16)

    cbarr = np.zeros((128, NB, 128), f32)
    for n in range(NB):
        cbarr[:, n, :] = np.tile(c_batch[:, n], 4)[None, :]
    cbarr = cbarr.reshape(128, NB * 128).astype(bf16)

    in_maps = []
    for core in range(NCORES):
        d, a = core // 4, core % 4
        # gate col index per (j, tg, c''): OFT[tg] + 128a + 32j + c''
        gidx = np.empty((4, 4, 32), np.int64)
        for j in range(4):
            for tg in range(4):
                gidx[j, tg] = OFT[tg] + 128 * a + 32 * j + np.arange(32)
        gidx = gidx.reshape(512)

        xp = XP[d][:, :TSTEPS, gidx]                      # [B, TS, 512]
        xp = np.ascontiguousarray(xp.transpose(1, 0, 2)).astype(bf16)

        W = WHH[d]                                        # [NB, G, C]
        wt = np.empty((128, NB * 4 * 512), f32)
        for n in range(NB):
            for dlt in range(4):
                blk = a ^ dlt
                wt[:, (n * 4 + dlt) * 512:(n * 4 + dlt + 1) * 512] = \
                    W[n][gidx][:, 128 * blk:128 * blk + 128].T
        wt = wt.astype(bf16)

        vrep = np.zeros((128, TSTEPS), f32)
        for j in range(4):
            vrep[32 * j:32 * j + 32] = VAL[d][:, :TSTEPS]

        in_maps.append({
            "wt": wt,
            "xp": xp,
            "cb": cbarr,
            "vt": vrep,
            "idq": idq,
        })
    return in_maps


def _assemble(results, TSTEPS=T):
    out = np.empty((B, TSTEPS, 2 * C), np.float32)
    for core in range(NCORES):
        d, a = core // 4, core % 4
        ho = np.asarray(results[core]["ho"], np.float32)  # [128, TS*32]
        ho = ho.reshape(4, 32, TSTEPS, 32)                # [j, s, t, c']
        ho = ho.transpose(1, 2, 0, 3).reshape(32, TSTEPS, 128)  # [s, t, cell]
        if d == 1:
            ho = ho[:, ::-1]
        out[:, :, d * 512 + 128 * a: d * 512 + 128 * a + 128] = ho
    return out


def kernel(**inputs):
    from concourse.bass_utils import run_bass_kernel_spmd

    in_maps = _host_prep(**inputs)
    if "nc" not in _CACHE:
        _CACHE["nc"] = _build_program()
    res = run_bass_kernel_spmd(_CACHE["nc"], in_maps, list(range(NCORES)))
    return _assemble(res.results)


# revision 16
# speedup vs baseline: 2.1291x; 1.1083x over previous
"""BasisCustBiLSTM Trainium2 kernel — cross-core-sharded recurrence.

Host: metadata MLP -> c_batch; per-sample mixed input projections XP
(BLAS); per-core packing. Device: 8 cores = 2 directions x 4 cell-blocks
of 128 cells. Each core computes the gates for its 128 cells for ALL 32
samples (M=32 stationary columns; the basis-weight stream is shared by
the whole batch, 4x less streaming than group-sharded layouts), then
exchanges its h-block with its 3 same-direction peers every step via
remote SBUF-to-SBUF DMA (XOR-relative routing, lane-merged broadcasts).

Per step: 4 col-group PE chains (one per 32-cell quarter, 128 gate cols
[i|f|o|g], 33 matmuls N=128 each, concurrent); fused epilogue over
[128 = 4 quarters x 32 samples, 32] tiles; 4 concurrent 32x32 PE
transposes -> hT block; broadcast to quad peers; 8 DVE muls build the
c-scaled stationaries (A-trick) for the next step.

Tile's scheduler cannot model remote semaphores, so the program is built
without the cross-core waits; they are attached to nofuse NOPs after
scheduling, with no-sync edges enforcing engine order.
"""

import sys

for p in ("/opt/trn_rl_repo",):
    if p not in sys.path:
        sys.path.insert(0, p)

import numpy as np
import ml_dtypes

B, T, I, C = 32, 256, 512, 512
G = 4 * C
NB, EMB, KQ = 8, 64, 64
NCORES = 8
OFT = (0, 512, 1536, 1024)  # gate-type col offsets in [i|f|g|o] space for [i|f|o|g] packing

bf16 = ml_dtypes.bfloat16

_CACHE = {}


def _build_program(TSTEPS=T):
    import concourse.bass as bass
    import concourse.mybir as mybir
    from concourse import bacc, tile
    from concourse.tile_rust import add_dep_helper

    dt = mybir.dt
    AF = mybir.ActivationFunctionType

    nc = bacc.Bacc(None, target_bir_lowering=False, num_swdge_queues=4)

    wt_d = nc.dram_tensor("wt", [128, NB * 4 * 512], dt.bfloat16, kind="ExternalInput")
    xp_d = nc.dram_tensor("xp", [TSTEPS, 32, 512], dt.bfloat16, kind="ExternalInput")
    cb_d = nc.dram_tensor("cb", [128, NB * 128], dt.bfloat16, kind="ExternalInput")
    vt_d = nc.dram_tensor("vt", [128, TSTEPS], dt.float32, kind="ExternalInput")
    id_d = nc.dram_tensor("idq", [128, 32], dt.bfloat16, kind="ExternalInput")
    ho_d = nc.dram_tensor("ho", [128, TSTEPS * 32], dt.bfloat16, kind="ExternalOutput")

    post_waits = []

    with tile.TileContext(nc) as tc:
        with (
            tc.tile_pool(name="wt", bufs=1) as wt_pool,
            tc.tile_pool(name="const", bufs=1) as const_pool,
            tc.tile_pool(name="state", bufs=1) as state_pool,
            tc.tile_pool(name="xp", bufs=3) as xp_pool,
            tc.tile_pool(name="hn", bufs=2) as hn_pool,
            tc.tile_pool(name="hn0", bufs=1) as hn0_pool,
            tc.tile_pool(name="scr", bufs=2) as scr_pool,
            tc.tile_pool(name="ps", bufs=2, space="PSUM") as ps_pool,
            tc.tile_pool(name="pst", bufs=2, space="PSUM") as pst_pool,
        ):
            wt = wt_pool.tile([128, NB * 4 * 512], dt.bfloat16)
            stripe = NB * 4 * 512 // 8
            for j in range(8):
                nc.sync.dma_start(
                    wt[:, j * stripe:(j + 1) * stripe],
                    wt_d[:, j * stripe:(j + 1) * stripe],
                )

            cb = const_pool.tile([128, NB * 128], dt.bfloat16, tag="cb")
            nc.sync.dma_start(cb[:], cb_d[:])
            vt = const_pool.tile([128, TSTEPS], dt.float32, tag="vt")
            nc.sync.dma_start(vt[:], vt_d[:])
            idq = const_pool.tile([128, 32], dt.bfloat16, tag="idq")
            nc.sync.dma_start(idq[:], id_d[:])

            hosb = state_pool.tile([128, TSTEPS * 32], dt.bfloat16, tag="hosb")
            land = [state_pool.tile([128, 128], dt.bfloat16, name=f"land{i}")
                    for i in range(3)]
            gc = state_pool.tile([128, 64], dt.float32, tag="gc")  # [tanh(g) | cst]
            nc.vector.memset(gc[:], 0)
            awt = state_pool.tile([32, 1], dt.float32, tag="awt")

            hn0 = []
            for n in range(NB):
                t_ = hn0_pool.tile([128, 128], dt.bfloat16, name=f"hn0_{n}")
                nc.vector.memset(t_[:], 0)
                hn0.append(t_)

            rsems = {dlt: nc.alloc_semaphore(f"rsem{dlt}") for dlt in (1, 2, 3)}
            lsem = nc.alloc_semaphore("lsem")
            bsem = nc._bir_kernel_barrier_sem
            nc._bir_kernel_barrier_sem_replica_groups.append(set(range(NCORES)))

            barnop = nc.gpsimd.nop(hint="barrier", nofuse=True)
            post_waits.append((barnop, bsem, 1))

            prev_g = barnop
            prev_v = None
            hn_cur = hn0

            warm_ps = pst_pool.tile([32, 32], dt.float32, tag="warm", bufs=1)

            def warmer(dep=None):
                w = nc.tensor.matmul(
                    warm_ps[:], idq[0:32, :], idq[0:32, :],
                    start=True, stop=True, tile_position=(0, 0),
                )
                if dep is not None:
                    add_dep_helper(w.ins, dep.ins, sync=True, reason="warm spacing")
                return w

            def emit_seed_own(gates, xpt, hn):
                """Seed matmuls + own-chunk (dlt=0) accumulation."""
                for j in range(4):
                    nc.tensor.matmul(
                        gates[32 * j:32 * j + 32, :],
                        idq[0:32, :], xpt[:, 128 * j:128 * j + 128],
                        start=True, stop=False, tile_position=(0, 32 * j),
                    )
                for n in range(NB):
                    col0 = (n * 4) * 512
                    for j in range(4):
                        nc.tensor.matmul(
                            gates[32 * j:32 * j + 32, :],
                            hn[n][:, 0:32],
                            wt[:, col0 + 128 * j:col0 + 128 * j + 128],
                            start=False, stop=False, tile_position=(0, 32 * j),
                        )

            def emit_remote(gates, hn):
                """Remote-chunk (dlt=1..3) accumulation, closing the chains.
                dlt-outer so each chunk's matmuls run as it arrives."""
                for dlt in (1, 2, 3):
                    for n in range(NB):
                        col0 = (n * 4 + dlt) * 512
                        last = (n == NB - 1 and dlt == 3)
                        for j in range(4):
                            nc.tensor.matmul(
                                gates[32 * j:32 * j + 32, :],
                                hn[n][:, 32 * dlt:32 * dlt + 32],
                                wt[:, col0 + 128 * j:col0 + 128 * j + 128],
                                start=False, stop=last, tile_position=(0, 32 * j),
                            )

            # prologue: step 0 computes gates entirely from xp (hn0 == 0)
            xpt = xp_pool.tile([32, 512], dt.bfloat16, tag="xpt")
            nc.sync.dma_start(xpt[:], xp_d[0, :, :])
            gates_cur = ps_pool.tile([128, 128], dt.float32, tag="gates")
            emit_seed_own(gates_cur, xpt, hn0)

            for t in range(TSTEPS):
                # hoisted descriptor generation for this step's broadcasts
                # (prepare-only: no data read; triggers come after the copy)
                if t < TSTEPS - 1:
                    lnd = land[t % 3]
                    for dlt in (1, 2, 3):
                        p = nc.gpsimd.remote_dma_broadcast(
                            lnd[:, 32 * dlt:32 * dlt + 32], lnd[:, 0:32],
                            remote_sem=rsems[dlt], local_sem=lsem,
                            rdests=[(0, dlt)] * 8,
                        )
                        add_dep_helper(p.ins, prev_g.ins, sync=False, reason="gp order")
                        prev_g = p

                gates = gates_cur
                emit_remote(gates, hn_cur)

                # ---- epilogue (dense [128, *]) ----
                sigs = scr_pool.tile([128, 96], dt.float32, tag="sigs")
                nc.scalar.activation(gc[:, 0:32], gates[:, 96:128], AF.Tanh)
                nc.scalar.activation(sigs[:], gates[:, 0:96], AF.Sigmoid)
                prod = scr_pool.tile([128, 64], dt.float32, tag="prod")
                nc.vector.tensor_mul(prod[:], sigs[:, 0:64], gc[:])
                u = scr_pool.tile([128, 32], dt.float32, tag="u")
                nc.vector.tensor_add(u[:], prod[:, 0:32], prod[:, 32:64])
                nc.vector.tensor_scalar_mul(gc[:, 32:64], u[:], vt[:, t:t + 1])
                tcn = scr_pool.tile([128, 32], dt.float32, tag="tcn")
                atc = nc.scalar.activation(tcn[:], gc[:, 32:64], AF.Tanh)
                hmb = hosb[:, 32 * t:32 * t + 32]
                hop = nc.vector.tensor_mul(hmb, sigs[:, 64:96], tcn[:])
                warmer(atc)  # keep HAM warm through the epilogue

                if t % 32 == 31 or t == TSTEPS - 1:
                    lo = (t // 32) * 32
                    nc.sync.dma_start(
                        ho_d[:, lo * 32:(t + 1) * 32],
                        hosb[:, lo * 32:(t + 1) * 32],
                    )

                if t == TSTEPS - 1:
                    break

                # ---- transpose h block -> hT [cell, sample] ----
                hTps = pst_pool.tile([128, 32], dt.bfloat16, tag="hTps")
                for q in range(4):
                    nc.tensor.transpose(
                        hTps[32 * q:32 * q + 32, :],
                        hosb[32 * q:32 * q + 32, 32 * t:32 * t + 32],
                        idq[32 * q:32 * q + 32, :],
                        tile_position=(32 * q, 32 * q),
                    )

                lnd = land[t % 3]
                if t >= 3:
                    lw = nc.vector.nop(hint=f"lw{t}", nofuse=True)
                    post_waits.append((lw, lsem, 48 * (t - 2)))
                    if prev_v is not None:
                        add_dep_helper(lw.ins, prev_v.ins, sync=False, reason="vec order")
                    prev_v = lw
                cpy = nc.vector.tensor_copy(lnd[:, 0:32], hTps[:])
                if prev_v is not None:
                    add_dep_helper(cpy.ins, prev_v.ins, sync=False, reason="vec order")
                prev_v = cpy

                trig = nc.gpsimd.trigger_dma(count=None)
                add_dep_helper(trig.ins, prev_g.ins, sync=False, reason="gp order")
                add_dep_helper(trig.ins, cpy.ins, sync=True,
                               reason="hoisted preps: fire after h copy lands")
                prev_g = trig

                # ACT instruction-stream warmer: fires mid-exchange so the
                # next epilogue's activations don't pay an I-fetch stall.
                aw = nc.scalar.activation(awt[:], gc[0:32, 0:1], AF.Tanh)
                add_dep_helper(aw.ins, cpy.ins, sync=True, reason="ACT warm")

                # own-chunk stationaries + next step's seeds/own matmuls run
                # during the exchange; remote stationaries per arriving chunk.
                hn_nxt = []
                for n in range(NB):
                    t_ = hn_pool.tile([128, 128], dt.bfloat16, tag=f"hn{n}")
                    hn_nxt.append(t_)
                for n in range(NB):
                    mm = nc.vector.tensor_mul(
                        hn_nxt[n][:, 0:32], lnd[:, 0:32], cb[:, 128 * n:128 * n + 32])
                    add_dep_helper(mm.ins, prev_v.ins, sync=False, reason="vec order")
                    prev_v = mm

                xpt = xp_pool.tile([32, 512], dt.bfloat16, tag="xpt")
                nc.sync.dma_start(xpt[:], xp_d[t + 1, :, :])
                gates_cur = ps_pool.tile([128, 128], dt.float32, tag="gates")
                emit_seed_own(gates_cur, xpt, hn_nxt)

                for dlt in (1, 2, 3):
                    hwn = nc.vector.nop(hint=f"hw{t}_{dlt}", nofuse=True)
                    post_waits.append((hwn, rsems[dlt], 16 * (t + 1)))
                    add_dep_helper(hwn.ins, prev_v.ins, sync=False, reason="vec order")
                    prev_v = hwn
                    for n in range(NB):
                        mm = nc.vector.tensor_mul(
                            hn_nxt[n][:, 32 * dlt:32 * dlt + 32],
                            lnd[:, 32 * dlt:32 * dlt + 32],
                            cb[:, 128 * n + 32 * dlt:128 * n + 32 * dlt + 32])
                        add_dep_helper(mm.ins, prev_v.ins, sync=False, reason="vec order")
                        prev_v = mm
                hn_cur = hn_nxt

    for inst, sem, val in post_waits:
        inst._wait_ge(sem, val)

    nc.finalize()
    nc.insert_bir_kernel_barrier_sem_inc()
    return nc


def _host_prep(x, mask, meta_author, meta_century, emb_author, emb_century,
               P_W1, P_b1, P_W2, W_ih, W_hh, b, W_ih_rev, W_hh_rev, b_rev,
               TSTEPS=T):
    f32 = np.float32
    x = np.asarray(x, f32)
    mask = np.asarray(mask)
    q = np.concatenate(
        [np.asarray(emb_author, f32)[np.asarray(meta_author)],
         np.asarray(emb_century, f32)[np.asarray(meta_century)]], axis=1)
    h1 = np.tanh(q @ np.asarray(P_W1, f32) + np.asarray(P_b1, f32))
    logits = h1 @ np.asarray(P_W2, f32)
    e = np.exp(logits - logits.max(axis=1, keepdims=True))
    c_batch = (e / e.sum(axis=1, keepdims=True)).astype(f32)

    lengths = mask.astype(np.int64).sum(axis=1)
    t = np.arange(T)
    valid_f = (t[None, :] < lengths[:, None]).astype(f32)
    valid_r = ((T - t)[None, :] <= lengths[:, None]).astype(f32)

    def xproj(Wb, bb, xs):
        Wm = np.tensordot(c_batch, np.asarray(Wb, f32), axes=([1], [0]))
        bm = c_batch @ np.asarray(bb, f32)
        out = np.empty((B, T, G), f32)
        for i in range(B):
            np.matmul(xs[i], Wm[i].T, out=out[i])
        out += bm[:, None, :]
        return out  # [B, T, G] natural [i|f|g|o]

    x_rev = x[:, ::-1]
    XP = [xproj(W_ih, b, x), xproj(W_ih_rev, b_rev, x_rev)]
    WHH = [np.asarray(W_hh, f32), np.asarray(W_hh_rev, f32)]
    VAL = [valid_f, valid_r]

    idq = np.zeros((128, 32), dtype=bf16)
    for qq in range(4):
        idq[32 * qq:32 * qq + 32] = np.eye(32, dtype=bf16)

    cbarr = np.zeros((128, NB, 128), f32)
    for n in range(NB):
        cbarr[:, n, :] = np.tile(c_batch[:, n], 4)[None, :]
    cbarr = cbarr.reshape(128, NB * 128).astype(bf16)

    in_maps = []
    for core in range(NCORES):
        d, a = core // 4, core % 4
        # gate col index per (j, tg, c''): OFT[tg] + 128a + 32j + c''
        gidx = np.empty((4, 4, 32), np.int64)
        for j in range(4):
            for tg in range(4):
                gidx[j, tg] = OFT[tg] + 128 * a + 32 * j + np.arange(32)
        gidx = gidx.reshape(512)

        xp = XP[d][:, :TSTEPS, gidx]                      # [B, TS, 512]
        xp = np.ascontiguousarray(xp.transpose(1, 0, 2)).astype(bf16)

        W = WHH[d]                                        # [NB, G, C]
        wt = np.empty((128, NB * 4 * 512), f32)
        for n in range(NB):
            for dlt in range(4):
                blk = a ^ dlt
                wt[:, (n * 4 + dlt) * 512:(n * 4 + dlt + 1) * 512] = \
                    W[n][gidx][:, 128 * blk:128 * blk + 128].T
        wt = wt.astype(bf16)

        vrep = np.zeros((128, TSTEPS), f32)
        for j in range(4):
            vrep[32 * j:32 * j + 32] = VAL[d][:, :TSTEPS]

        in_maps.append({
            "wt": wt,
            "xp": xp,
            "cb": cbarr,
            "vt": vrep,
            "idq": idq,
        })
    return in_maps


def _assemble(results, TSTEPS=T):
    out = np.empty((B, TSTEPS, 2 * C), np.float32)
    for core in range(NCORES):
        d, a = core // 4, core % 4
        ho = np.asarray(results[core]["ho"], np.float32)  # [128, TS*32]
        ho = ho.reshape(4, 32, TSTEPS, 32)                # [j, s, t, c']
        ho = ho.transpose(1, 2, 0, 3).reshape(32, TSTEPS, 128)  # [s, t, cell]
        if d == 1:
            ho = ho[:, ::-1]
        out[:, :, d * 512 + 128 * a: d * 512 + 128 * a + 128] = ho
    return out


def kernel(**inputs):
    from concourse.bass_utils import run_bass_kernel_spmd

    in_maps = _host_prep(**inputs)
    if "nc" not in _CACHE:
        _CACHE["nc"] = _build_program()
    res = run_bass_kernel_spmd(_CACHE["nc"], in_maps, list(range(NCORES)))
    return _assemble(res.results)


# revision 17
# speedup vs baseline: 2.4350x; 1.1437x over previous
"""BasisCustBiLSTM Trainium2 kernel — cross-core-sharded recurrence.

Host: metadata MLP -> c_batch; per-sample mixed input projections XP
(BLAS); per-core packing. Device: 8 cores = 2 directions x 4 cell-blocks
of 128 cells. Each core computes the gates for its 128 cells for ALL 32
samples (M=32 stationary columns; the basis-weight stream is shared by
the whole batch, 4x less streaming than group-sharded layouts), then
exchanges its h-block with its 3 same-direction peers every step via
remote SBUF-to-SBUF DMA (XOR-relative routing, lane-merged broadcasts).

Per step: 4 col-group PE chains (one per 32-cell quarter, 128 gate cols
[i|f|o|g], 33 matmuls N=128 each, concurrent); fused epilogue over
[128 = 4 quarters x 32 samples, 32] tiles; 4 concurrent 32x32 PE
transposes -> hT block; broadcast to quad peers; 8 DVE muls build the
c-scaled stationaries (A-trick) for the next step.

Tile's scheduler cannot model remote semaphores, so the program is built
without the cross-core waits; they are attached to nofuse NOPs after
scheduling, with no-sync edges enforcing engine order.
"""

import sys

for p in ("/opt/trn_rl_repo",):
    if p not in sys.path:
        sys.path.insert(0, p)

import numpy as np
import ml_dtypes

B, T, I, C = 32, 256, 512, 512
G = 4 * C
NB, EMB, KQ = 8, 64, 64
NCORES = 8
OFT = (0, 512, 1536, 1024)  # gate-type col offsets in [i|f|g|o] space for [i|f|o|g] packing

bf16 = ml_dtypes.bfloat16

_CACHE = {}


def _build_program(TSTEPS=T):
    import concourse.bass as bass
    import concourse.mybir as mybir
    from concourse import bacc, tile
    from concourse.tile_rust import add_dep_helper

    dt = mybir.dt
    AF = mybir.ActivationFunctionType

    nc = bacc.Bacc(None, target_bir_lowering=False, num_swdge_queues=4)

    wt_d = nc.dram_tensor("wt", [128, NB * 4 * 512], dt.bfloat16, kind="ExternalInput")
    xp_d = nc.dram_tensor("xp", [TSTEPS, 32, 512], dt.bfloat16, kind="ExternalInput")
    cb_d = nc.dram_tensor("cb", [128, NB * 128], dt.bfloat16, kind="ExternalInput")
    vt_d = nc.dram_tensor("vt", [128, TSTEPS], dt.float32, kind="ExternalInput")
    id_d = nc.dram_tensor("idq", [128, 32], dt.bfloat16, kind="ExternalInput")
    ho_d = nc.dram_tensor("ho", [128, TSTEPS * 32], dt.bfloat16, kind="ExternalOutput")

    post_waits = []

    with tile.TileContext(nc) as tc:
        with (
            tc.tile_pool(name="wt", bufs=1) as wt_pool,
            tc.tile_pool(name="const", bufs=1) as const_pool,
            tc.tile_pool(name="state", bufs=1) as state_pool,
            tc.tile_pool(name="xp", bufs=3) as xp_pool,
            tc.tile_pool(name="hn", bufs=2) as hn_pool,
            tc.tile_pool(name="hn0", bufs=1) as hn0_pool,
            tc.tile_pool(name="scr", bufs=2) as scr_pool,
            tc.tile_pool(name="ps", bufs=2, space="PSUM") as ps_pool,
            tc.tile_pool(name="pst", bufs=2, space="PSUM") as pst_pool,
        ):
            wt = wt_pool.tile([128, NB * 4 * 512], dt.bfloat16)
            stripe = NB * 4 * 512 // 8
            for j in range(8):
                nc.sync.dma_start(
                    wt[:, j * stripe:(j + 1) * stripe],
                    wt_d[:, j * stripe:(j + 1) * stripe],
                )

            cb = const_pool.tile([128, NB * 128], dt.bfloat16, tag="cb")
            nc.sync.dma_start(cb[:], cb_d[:])
            vt = const_pool.tile([128, TSTEPS], dt.float32, tag="vt")
            nc.sync.dma_start(vt[:], vt_d[:])
            idq = const_pool.tile([128, 32], dt.bfloat16, tag="idq")
            nc.sync.dma_start(idq[:], id_d[:])

            hosb = state_pool.tile([128, TSTEPS * 32], dt.bfloat16, tag="hosb")
            land = [state_pool.tile([128, 128], dt.bfloat16, name=f"land{i}")
                    for i in range(3)]
            gc = state_pool.tile([128, 64], dt.float32, tag="gc")  # [tanh(g) | cst]
            nc.vector.memset(gc[:], 0)
            awt = state_pool.tile([32, 1], dt.float32, tag="awt")

            hn0 = []
            for n in range(NB):
                t_ = hn0_pool.tile([128, 128], dt.bfloat16, name=f"hn0_{n}")
                nc.vector.memset(t_[:], 0)
                hn0.append(t_)

            rsems = {dlt: nc.alloc_semaphore(f"rsem{dlt}") for dlt in (1, 2, 3)}
            lsem = nc.alloc_semaphore("lsem")
            bsem = nc._bir_kernel_barrier_sem
            nc._bir_kernel_barrier_sem_replica_groups.append(set(range(NCORES)))

            barnop = nc.gpsimd.nop(hint="barrier", nofuse=True)
            post_waits.append((barnop, bsem, 1))

            prev_g = barnop
            prev_v = None
            hn_cur = hn0

            warm_ps = pst_pool.tile([32, 32], dt.float32, tag="warm", bufs=1)

            def warmer(dep=None):
                w = nc.tensor.matmul(
                    warm_ps[:], idq[0:32, :], idq[0:32, :],
                    start=True, stop=True, tile_position=(0, 0),
                )
                if dep is not None:
                    add_dep_helper(w.ins, dep.ins, sync=True, reason="warm spacing")
                return w

            def emit_seed_own(gates, xpt, hn):
                """Seed matmuls + own-chunk (dlt=0) accumulation."""
                for j in range(4):
                    nc.tensor.matmul(
                        gates[32 * j:32 * j + 32, :],
                        idq[0:32, :], xpt[:, 128 * j:128 * j + 128],
                        start=True, stop=False, tile_position=(0, 32 * j),
                    )
                for n in range(NB):
                    col0 = (n * 4) * 512
                    for j in range(4):
                        nc.tensor.matmul(
                            gates[32 * j:32 * j + 32, :],
                            hn[n][:, 0:32],
                            wt[:, col0 + 128 * j:col0 + 128 * j + 128],
                            start=False, stop=False, tile_position=(0, 32 * j),
                        )

            def emit_remote(gates, hn):
                """Remote-chunk (dlt=1..3) accumulation, closing the chains.
                dlt-outer so each chunk's matmuls run as it arrives."""
                for dlt in (1, 2, 3):
                    for n in range(NB):
                        col0 = (n * 4 + dlt) * 512
                        last = (n == NB - 1 and dlt == 3)
                        for j in range(4):
                            nc.tensor.matmul(
                                gates[32 * j:32 * j + 32, :],
                                hn[n][:, 32 * dlt:32 * dlt + 32],
                                wt[:, col0 + 128 * j:col0 + 128 * j + 128],
                                start=False, stop=last, tile_position=(0, 32 * j),
                            )

            # prologue: step 0 computes gates entirely from xp (hn0 == 0)
            xpt = xp_pool.tile([32, 512], dt.bfloat16, tag="xpt")
            nc.sync.dma_start(xpt[:], xp_d[0, :, :])
            gates_cur = ps_pool.tile([128, 128], dt.float32, tag="gates")
            emit_seed_own(gates_cur, xpt, hn0)

            for t in range(TSTEPS):
                # hoisted descriptor generation for this step's broadcasts
                # (prepare-only: no data read; triggers come after the copy)
                if t < TSTEPS - 1:
                    lnd = land[t % 3]
                    for dlt in (1, 2, 3):
                        p = nc.gpsimd.remote_dma_broadcast(
                            lnd[:, 32 * dlt:32 * dlt + 32], lnd[:, 0:32],
                            remote_sem=rsems[dlt], local_sem=lsem,
                            rdests=[(0, dlt)] * 8,
                        )
                        add_dep_helper(p.ins, prev_g.ins, sync=False, reason="gp order")
                        prev_g = p

                gates = gates_cur
                emit_remote(gates, hn_cur)

                # ---- epilogue (dense [128, *]) ----
                sigs = scr_pool.tile([128, 96], dt.float32, tag="sigs")
                nc.scalar.activation(gc[:, 0:32], gates[:, 96:128], AF.Tanh)
                nc.scalar.activation(sigs[:], gates[:, 0:96], AF.Sigmoid)
                prod = scr_pool.tile([128, 64], dt.float32, tag="prod")
                nc.vector.tensor_mul(prod[:], sigs[:, 0:64], gc[:])
                u = scr_pool.tile([128, 32], dt.float32, tag="u")
                nc.vector.tensor_add(u[:], prod[:, 0:32], prod[:, 32:64])
                nc.vector.tensor_scalar_mul(gc[:, 32:64], u[:], vt[:, t:t + 1])
                tcn = scr_pool.tile([128, 32], dt.float32, tag="tcn")
                atc = nc.scalar.activation(tcn[:], gc[:, 32:64], AF.Tanh)
                hmb = hosb[:, 32 * t:32 * t + 32]
                hop = nc.vector.tensor_mul(hmb, sigs[:, 64:96], tcn[:])
                warmer(atc)  # keep HAM warm through the epilogue

                if t % 32 == 31 or t == TSTEPS - 1:
                    lo = (t // 32) * 32
                    nc.sync.dma_start(
                        ho_d[:, lo * 32:(t + 1) * 32],
                        hosb[:, lo * 32:(t + 1) * 32],
                    )

                if t == TSTEPS - 1:
                    break

                # ---- transpose h block -> hT [cell, sample] ----
                hTps = pst_pool.tile([128, 32], dt.bfloat16, tag="hTps")
                for q in range(4):
                    nc.tensor.transpose(
                        hTps[32 * q:32 * q + 32, :],
                        hosb[32 * q:32 * q + 32, 32 * t:32 * t + 32],
                        idq[32 * q:32 * q + 32, :],
                        tile_position=(32 * q, 32 * q),
                    )

                lnd = land[t % 3]
                if t >= 3:
                    lw = nc.vector.nop(hint=f"lw{t}", nofuse=True)
                    post_waits.append((lw, lsem, 48 * (t - 2)))
                    if prev_v is not None:
                        add_dep_helper(lw.ins, prev_v.ins, sync=False, reason="vec order")
                    prev_v = lw
                cpy = nc.vector.tensor_copy(lnd[:, 0:32], hTps[:])
                if prev_v is not None:
                    add_dep_helper(cpy.ins, prev_v.ins, sync=False, reason="vec order")
                prev_v = cpy

                trig = nc.gpsimd.trigger_dma(count=None)
                add_dep_helper(trig.ins, prev_g.ins, sync=False, reason="gp order")
                add_dep_helper(trig.ins, cpy.ins, sync=True,
                               reason="hoisted preps: fire after h copy lands")
                prev_g = trig

                # ACT instruction-stream warmer: fires mid-exchange so the
                # next epilogue's activations don't pay an I-fetch stall.
                aw = nc.scalar.activation(awt[:], gc[0:32, 0:1], AF.Tanh)
                add_dep_helper(aw.ins, cpy.ins, sync=True, reason="ACT warm")

                # own-chunk stationaries + next step's seeds/own matmuls run
                # during the exchange; remote stationaries per arriving chunk.
                hn_nxt = []
                for n in range(NB):
                    t_ = hn_pool.tile([128, 128], dt.bfloat16, tag=f"hn{n}")
                    hn_nxt.append(t_)
                for n in range(NB):
                    mm = nc.vector.tensor_mul(
                        hn_nxt[n][:, 0:32], lnd[:, 0:32], cb[:, 128 * n:128 * n + 32])
                    add_dep_helper(mm.ins, prev_v.ins, sync=False, reason="vec order")
                    prev_v = mm

                xpt = xp_pool.tile([32, 512], dt.bfloat16, tag="xpt")
                nc.sync.dma_start(xpt[:], xp_d[t + 1, :, :])
                gates_cur = ps_pool.tile([128, 128], dt.float32, tag="gates")
                emit_seed_own(gates_cur, xpt, hn_nxt)

                for dlt in (1, 2, 3):
                    hwn = nc.vector.nop(hint=f"hw{t}_{dlt}", nofuse=True)
                    # All 8 slot-transfers of a broadcast write identical data;
                    # lanes (i, i+8) split the partition range the same way in
                    # every slot, so any 9 of the 16 lane-increments guarantee
                    # both halves landed — skip the straggler-lane tail.
                    post_waits.append((hwn, rsems[dlt], 16 * t + 9))
                    add_dep_helper(hwn.ins, prev_v.ins, sync=False, reason="vec order")
                    prev_v = hwn
                    if dlt == 1:
                        aw2 = nc.scalar.activation(awt[:], gc[0:32, 0:1], AF.Tanh)
                        add_dep_helper(aw2.ins, hwn.ins, sync=True,
                                       reason="ACT warm near epilogue")
                    for n in range(NB):
                        mm = nc.vector.tensor_mul(
                            hn_nxt[n][:, 32 * dlt:32 * dlt + 32],
                            lnd[:, 32 * dlt:32 * dlt + 32],
                            cb[:, 128 * n + 32 * dlt:128 * n + 32 * dlt + 32])
                        add_dep_helper(mm.ins, prev_v.ins, sync=False, reason="vec order")
                        prev_v = mm
                hn_cur = hn_nxt

    for inst, sem, val in post_waits:
        inst._wait_ge(sem, val)

    nc.finalize()
    nc.insert_bir_kernel_barrier_sem_inc()
    return nc


def _host_prep(x, mask, meta_author, meta_century, emb_author, emb_century,
               P_W1, P_b1, P_W2, W_ih, W_hh, b, W_ih_rev, W_hh_rev, b_rev,
               TSTEPS=T):
    f32 = np.float32
    x = np.asarray(x, f32)
    mask = np.asarray(mask)
    q = np.concatenate(
        [np.asarray(emb_author, f32)[np.asarray(meta_author)],
         np.asarray(emb_century, f32)[np.asarray(meta_century)]], axis=1)
    h1 = np.tanh(q @ np.asarray(P_W1, f32) + np.asarray(P_b1, f32))
    logits = h1 @ np.asarray(P_W2, f32)
    e = np.exp(logits - logits.max(axis=1, keepdims=True))
    c_batch = (e / e.sum(axis=1, keepdims=True)).astype(f32)

    lengths = mask.astype(np.int64).sum(axis=1)
    t = np.arange(T)
    valid_f = (t[None, :] < lengths[:, None]).astype(f32)
    valid_r = ((T - t)[None, :] <= lengths[:, None]).astype(f32)

    def xproj(Wb, bb, xs):
        Wm = np.tensordot(c_batch, np.asarray(Wb, f32), axes=([1], [0]))
        bm = c_batch @ np.asarray(bb, f32)
        out = np.empty((B, T, G), f32)
        for i in range(B):
            np.matmul(xs[i], Wm[i].T, out=out[i])
        out += bm[:, None, :]
        return out  # [B, T, G] natural [i|f|g|o]

    x_rev = x[:, ::-1]
    XP = [xproj(W_ih, b, x), xproj(W_ih_rev, b_rev, x_rev)]
    WHH = [np.asarray(W_hh, f32), np.asarray(W_hh_rev, f32)]
    VAL = [valid_f, valid_r]

    idq = np.zeros((128, 32), dtype=bf16)
    for qq in range(4):
        idq[32 * qq:32 * qq + 32] = np.eye(32, dtype=bf16)

    cbarr = np.zeros((128, NB, 128), f32)
    for n in range(NB):
        cbarr[:, n, :] = np.tile(c_batch[:, n], 4)[None, :]
    cbarr = cbarr.reshape(128, NB * 128).astype(bf16)

    in_maps = []
    for core in range(NCORES):
        d, a = core // 4, core % 4
        # gate col index per (j, tg, c''): OFT[tg] + 128a + 32j + c''
        gidx = np.empty((4, 4, 32), np.int64)
        for j in range(4):
            for tg in range(4):
                gidx[j, tg] = OFT[tg] + 128 * a + 32 * j + np.arange(32)
        gidx = gidx.reshape(512)

        xp = XP[d][:, :TSTEPS, gidx]                      # [B, TS, 512]
        xp = np.ascontiguousarray(xp.transpose(1, 0, 2)).astype(bf16)

        W = WHH[d]                                        # [NB, G, C]
        wt = np.empty((128, NB * 4 * 512), f32)
        for n in range(NB):
            for dlt in range(4):
                blk = a ^ dlt
                wt[:, (n * 4 + dlt) * 512:(n * 4 + dlt + 1) * 512] = \
                    W[n][gidx][:, 128 * blk:128 * blk + 128].T
        wt = wt.astype(bf16)

        vrep = np.zeros((128, TSTEPS), f32)
        for j in range(4):
            vrep[32 * j:32 * j + 32] = VAL[d][:, :TSTEPS]

        in_maps.append({
            "wt": wt,
            "xp": xp,
            "cb": cbarr,
            "vt": vrep,
            "idq": idq,
        })
    return in_maps


def _assemble(results, TSTEPS=T):
    out = np.empty((B, TSTEPS, 2 * C), np.float32)
    for core in range(NCORES):
        d, a = core // 4, core % 4
        ho = np.asarray(results[core]["ho"], np.float32)  # [128, TS*32]
        ho = ho.reshape(4, 32, TSTEPS, 32)                # [j, s, t, c']
        ho = ho.transpose(1, 2, 0, 3).reshape(32, TSTEPS, 128)  # [s, t, cell]
        if d == 1:
            ho = ho[:, ::-1]
        out[:, :, d * 512 + 128 * a: d * 512 + 128 * a + 128] = ho
    return out


def kernel(**inputs):
    from concourse.bass_utils import run_bass_kernel_spmd

    in_maps = _host_prep(**inputs)
    if "nc" not in _CACHE:
        _CACHE["nc"] = _build_program()
    res = run_bass_kernel_spmd(_CACHE["nc"], in_maps, list(range(NCORES)))
    return _assemble(res.results)
